# revision 5
# baseline (speedup 1.0000x reference)
"""Bass/Tile kernel for sparse_attention (nn_Attention_78889959293500) on 8 trn2 cores.

Math (per batch b):
    values = W @ word_features[b] + bias            # [128, 32]
    S[n, t] = sum_k image[b, k, n] * values[k, t]   # [16384, 32]
    S = where(words_mask, -inf, S)
    P = softmax(S, axis=t)                          # [16384, 32]  -> coeff output
    attn[k, n] = sum_t values[k, t] * P[n, t]       # [128, 16384] -> attn output

Sharding: data-parallel over batch B=32 across 8 cores (4 batches each).

Per-core pipeline (band = 512 columns of n):
    S^T [32, 512]   = matmul(lhsT=values [128,32], rhs=q [128,512])  (PE, fp32)
    E^T [32, 512]   = exp(S^T + maskbias[t])        (ACT; mask folded into the
                      per-partition bias; no max-subtraction: |S|max ~65 < 88)
    E   [128,4,32]  = PE transpose of E^T
    sums[128,4]     = reduce over t  (DVE), r = 1/sums (DVE)
    P   [128,4,32]  = E * r          (DVE)          -> DMA to coeff
    P^T [32, 512]   = PE transpose of P (fp32r)     -> SBUF via ACT copy
    U   [128, 512]  = matmul(lhsT=V^T, rhs=P^T) in fp32r (1cyc/row at N=512)
                     -> SBUF via DVE copy -> DMA to attn

The whole per-core body sits in a runtime-count For_i loop ("reps" input, =1 for
normal runs) so the dev harness can time N in-NEFF iterations back to back.
"""

import numpy as np

import concourse.bacc as bacc
import concourse.bass as bass
import concourse.tile as tile
from concourse import mybir
from concourse import bass_utils

# Problem shapes (hardcoded per the harness contract).
B, D, T = 32, 256, 32
Dh, N = 128, 128 * 128
NCORES = 8
BC = B // NCORES          # batches per core
NB = 512                  # n-band size
NBANDS = N // NB          # 32 bands per batch
QCHUNK = 4096             # image DMA chunk (elems of n)
F32 = mybir.dt.float32
F32R = mybir.dt.float32r
I32 = mybir.dt.int32


def build(loop=True):
    nc = bacc.Bacc("TRN2", target_bir_lowering=False, debug=False, num_devices=NCORES)

    img = nc.dram_tensor("img", (BC, Dh, N), F32, kind="ExternalInput")
    wf = nc.dram_tensor("wf", (BC, D, T), F32, kind="ExternalInput")
    wt = nc.dram_tensor("wt", (D, Dh), F32, kind="ExternalInput")        # W transposed
    bb = nc.dram_tensor("bb", (Dh, 1), F32, kind="ExternalInput")
    maskneg = nc.dram_tensor("maskneg", (BC, T), F32, kind="ExternalInput")
    ident = nc.dram_tensor("ident", (128, 128), F32, kind="ExternalInput")
    reps = nc.dram_tensor("reps", (1, 1), I32, kind="ExternalInput")

    attn = nc.dram_tensor("attn", (BC, Dh, N), F32, kind="ExternalOutput")
    coeff = nc.dram_tensor("coeff", (BC, N, T), F32, kind="ExternalOutput")

    with tile.TileContext(nc) as tc:
        with (
            tc.tile_pool(name="const", bufs=1) as const_pool,
            tc.tile_pool(name="qin", bufs=3) as q_pool,
            tc.tile_pool(name="etn", bufs=3) as etn_pool,
            tc.tile_pool(name="pnt", bufs=3) as pnt_pool,
            tc.tile_pool(name="ptsb", bufs=3) as ptsb_pool,
            tc.tile_pool(name="attnsb", bufs=3) as attnsb_pool,
            tc.tile_pool(name="small", bufs=4) as small_pool,
            tc.tile_pool(name="psA", bufs=2, space="PSUM") as psA,   # stn
            tc.tile_pool(name="psB", bufs=2, space="PSUM") as psB,   # ent
            tc.tile_pool(name="psC", bufs=1, space="PSUM") as psC,   # pt
            tc.tile_pool(name="psD", bufs=2, space="PSUM") as psD,   # u
            tc.tile_pool(name="psE", bufs=1, space="PSUM") as psE,   # batch setup
        ):
            # ---- per-core constants ----
            wt_sb = const_pool.tile([128, 2, Dh], F32)      # W^T tiles: (d%128, d//128, k)
            nc.sync.dma_start(out=wt_sb[:], in_=wt.ap().rearrange("(j p) k -> p j k", p=128))
            b_sb = const_pool.tile([128, 1], F32)
            nc.sync.dma_start(out=b_sb[:], in_=bb.ap())
            id_sb = const_pool.tile([128, 128], F32)
            nc.sync.dma_start(out=id_sb[:], in_=ident.ap())
            id_sb_r = const_pool.tile([128, 128], F32R)
            nc.sync.dma_start(out=id_sb_r[:], in_=ident.ap().bitcast(F32R))
            mneg_sb = const_pool.tile([T, BC], F32)         # bias per (t, b)
            nc.sync.dma_start(out=mneg_sb[:], in_=maskneg.ap().rearrange("b t -> t b"))
            wf_sb = const_pool.tile([128, BC, 2, T], F32)   # (d%128, b, d//128, t)
            nc.sync.dma_start(out=wf_sb[:], in_=wf.ap().rearrange("b (j p) t -> p b j t", p=128))
            reps_sb = const_pool.tile([1, 1], I32)
            nc.sync.dma_start(out=reps_sb[:], in_=reps.ap())
            n_reps = nc.values_load(reps_sb[0:1, 0:1], min_val=1, max_val=1 << 20,
                                    skip_runtime_bounds_check=True)

            import contextlib
            loop_cm = tc.For_i(0, n_reps, 1) if loop else contextlib.nullcontext()
            with loop_cm:
                for b in range(BC):
                    # ---- values = W @ wf[b] + bias ----
                    vals_ps = psE.tile([128, T], F32, tag="bsetup")
                    nc.tensor.matmul(vals_ps[:], wt_sb[:, 0, :], wf_sb[:, b, 0, :],
                                     start=True, stop=False)
                    nc.tensor.matmul(vals_ps[:], wt_sb[:, 1, :], wf_sb[:, b, 1, :],
                                     start=False, stop=True)
                    values_sb = small_pool.tile([128, T], F32, tag="values")
                    nc.scalar.activation(values_sb[:], vals_ps[:],
                                         mybir.ActivationFunctionType.Identity,
                                         bias=b_sb[:])
                    # V^T (fp32r) for the attn matmul
                    vt_ps = psE.tile([T, 128], F32, tag="bsetup")
                    nc.tensor.matmul(vt_ps[:], values_sb[:], id_sb[:],
                                     is_transpose=True, start=True, stop=True)
                    vt_sb = small_pool.tile([T, 128], F32R, tag="vt")
                    nc.scalar.copy(vt_sb[:], vt_ps[:])

                    for ci in range(N // QCHUNK):
                        q_sb = q_pool.tile([128, QCHUNK], F32)
                        nc.sync.dma_start(out=q_sb[:],
                                          in_=img.ap()[b, :, ci * QCHUNK:(ci + 1) * QCHUNK])
                        for bi in range(QCHUNK // NB):
                            band = ci * (QCHUNK // NB) + bi
                            n0 = band * NB
                            # ---- S^T = values^T(k,t) . q(k,n), fp32 ----
                            stn_ps = psA.tile([T, NB], F32)
                            nc.tensor.matmul(stn_ps[:], values_sb[:],
                                             q_sb[:, bi * NB:(bi + 1) * NB],
                                             start=True, stop=True)
                            # ---- E^T = exp(S^T + maskbias) ----
                            etn_sb = etn_pool.tile([T, NB], F32)
                            nc.scalar.activation(etn_sb[:], stn_ps[:],
                                                 mybir.ActivationFunctionType.Exp,
                                                 bias=mneg_sb[:, b:b + 1])
                            # ---- transpose E^T -> E [128, 4, 32] ----
                            ent_ps = psB.tile([128, 4, T], F32)
                            for j in range(4):
                                nc.tensor.matmul(ent_ps[:, j, :],
                                                 etn_sb[:, j * 128:(j + 1) * 128],
                                                 id_sb[0:T, 0:T],
                                                 is_transpose=True, start=True, stop=True)
                            # ---- row softmax ----
                            sums = small_pool.tile([128, 4, 1], F32, tag="sums")
                            nc.vector.reduce_sum(sums[:], ent_ps[:], axis=mybir.AxisListType.X)
                            r = small_pool.tile([128, 4, 1], F32, tag="recip")
                            nc.vector.reciprocal(r[:], sums[:])
                            pnt_sb = pnt_pool.tile([128, 4, T], F32R)
                            nc.vector.tensor_mul(pnt_sb[:], ent_ps[:],
                                                 r[:].to_broadcast((128, 4, T)))
                            nc.sync.dma_start(
                                out=coeff.ap()[b, n0:n0 + NB, :].rearrange(
                                    "(j p) t -> p j t", p=128),
                                in_=pnt_sb[:].bitcast(F32))
                            # ---- transpose P -> P^T [32, 512] (fp32r) ----
                            pt_ps = psC.tile([T, NB], F32R)
                            for j in range(4):
                                nc.tensor.matmul(pt_ps[:, j * 128:(j + 1) * 128],
                                                 pnt_sb[:, j, :], id_sb_r[:],
                                                 is_transpose=True, start=True, stop=True)
                            pt_sb = ptsb_pool.tile([T, NB], F32R)
                            nc.scalar.copy(pt_sb[:], pt_ps[:])
                            # ---- U = V^T . P^T (fp32r) -> attn ----
                            u_ps = psD.tile([128, NB], F32)
                            nc.tensor.matmul(u_ps[:], vt_sb[:], pt_sb[:],
                                             start=True, stop=True)
                            attn_sb = attnsb_pool.tile([128, NB], F32)
                            nc.vector.tensor_copy(attn_sb[:], u_ps[:])
                            nc.sync.dma_start(out=attn.ap()[b, :, n0:n0 + NB],
                                              in_=attn_sb[:])

    nc.compile()
    return nc


_NC_CACHE = None


def _get_nc():
    global _NC_CACHE
    if _NC_CACHE is None:
        _NC_CACHE = build()
    return _NC_CACHE


def _run(inputs, trace=False, reps=1):
    word_features = np.ascontiguousarray(np.asarray(inputs["word_features"], dtype=np.float32))
    image_features = np.ascontiguousarray(np.asarray(inputs["image_features"], dtype=np.float32))
    words_mask = np.asarray(inputs["words_mask"])
    W = np.asarray(inputs["W"], dtype=np.float32)
    b = np.asarray(inputs["b"], dtype=np.float32)

    img_full = image_features.reshape(B, Dh, N)
    maskneg_full = np.where(words_mask != 0, np.float32(-1e30), np.float32(0.0)).astype(np.float32)
    wt = np.ascontiguousarray(W.T)
    bb = np.ascontiguousarray(b.reshape(Dh, 1))
    ident = np.eye(128, dtype=np.float32)
    reps_arr = np.array([[reps]], dtype=np.int32)

    in_maps = []
    for c in range(NCORES):
        sl = slice(c * BC, (c + 1) * BC)
        in_maps.append({
            "img": np.ascontiguousarray(img_full[sl]),
            "wf": np.ascontiguousarray(word_features[sl]),
            "wt": wt,
            "bb": bb,
            "maskneg": np.ascontiguousarray(maskneg_full[sl]),
            "ident": ident,
            "reps": reps_arr,
        })

    nc = _get_nc()
    res = bass_utils.run_bass_kernel_spmd(nc, in_maps, core_ids=list(range(NCORES)),
                                          trace=trace)
    attn = np.concatenate([r["attn"] for r in res.results], axis=0)
    coeff = np.concatenate([r["coeff"] for r in res.results], axis=0)
    return (attn, coeff), res


def kernel(**inputs):
    (attn, coeff), _ = _run(inputs)
    return attn, coeff


# revision 6
# speedup vs baseline: 23.2758x; 23.2758x over previous
"""Bass/Tile kernel for sparse_attention (nn_Attention_78889959293500) on 8 trn2 cores.

Math (per batch b):
    values = W @ word_features[b] + bias            # [128, 32]
    S[n, t] = sum_k image[b, k, n] * values[k, t]   # [16384, 32]
    S = where(words_mask, -inf, S)
    P = softmax(S, axis=t)                          # [16384, 32]  -> coeff output
    attn[k, n] = sum_t values[k, t] * P[n, t]       # [128, 16384] -> attn output

Sharding: data-parallel over batch B=32 across 8 cores (4 batches each).

Per-core pipeline (band = 512 columns of n):
    S^T [32, 512]   = matmul(lhsT=values [128,32], rhs=q [128,512])  (PE, fp32)
    E^T [32, 512]   = exp(S^T + maskbias[t])        (ACT; mask folded into the
                      per-partition bias; no max-subtraction: |S|max ~65 < 88)
    E   [128,4,32]  = PE transpose of E^T
    sums[128,4]     = reduce over t  (DVE), r = 1/sums (DVE)
    P   [128,4,32]  = E * r          (DVE)          -> DMA to coeff
    P^T [32, 512]   = PE transpose of P (fp32r)     -> SBUF via ACT copy
    U   [128, 512]  = matmul(lhsT=V^T, rhs=P^T) in fp32r (1cyc/row at N=512)
                     -> SBUF via DVE copy -> DMA to attn

The whole per-core body sits in a runtime-count For_i loop ("reps" input, =1 for
normal runs) so the dev harness can time N in-NEFF iterations back to back.
"""

import numpy as np

import concourse.bacc as bacc
import concourse.bass as bass
import concourse.tile as tile
from concourse import mybir
from concourse import bass_utils

# Problem shapes (hardcoded per the harness contract).
B, D, T = 32, 256, 32
Dh, N = 128, 128 * 128
NCORES = 8
BC = B // NCORES          # batches per core
NB = 512                  # n-band size
NBANDS = N // NB          # 32 bands per batch
QCHUNK = 4096             # image DMA chunk (elems of n)
F32 = mybir.dt.float32
F32R = mybir.dt.float32r
I32 = mybir.dt.int32


def build(loop=True):
    nc = bacc.Bacc("TRN2", target_bir_lowering=False, debug=False, num_devices=NCORES)

    img = nc.dram_tensor("img", (BC, Dh, N), F32, kind="ExternalInput")
    wf = nc.dram_tensor("wf", (BC, D, T), F32, kind="ExternalInput")
    wt = nc.dram_tensor("wt", (D, Dh), F32, kind="ExternalInput")        # W transposed
    bb = nc.dram_tensor("bb", (Dh, 1), F32, kind="ExternalInput")
    maskneg = nc.dram_tensor("maskneg", (BC, T), F32, kind="ExternalInput")
    ident = nc.dram_tensor("ident", (128, 128), F32, kind="ExternalInput")
    reps = nc.dram_tensor("reps", (1, 1), I32, kind="ExternalInput")

    attn = nc.dram_tensor("attn", (BC, Dh, N), F32, kind="ExternalOutput")
    coeff = nc.dram_tensor("coeff", (BC, N, T), F32, kind="ExternalOutput")

    with tile.TileContext(nc) as tc:
        with (
            tc.tile_pool(name="const", bufs=1) as const_pool,
            tc.tile_pool(name="qin", bufs=3) as q_pool,
            tc.tile_pool(name="etn", bufs=3) as etn_pool,
            tc.tile_pool(name="pnt", bufs=3) as pnt_pool,
            tc.tile_pool(name="ptsb", bufs=3) as ptsb_pool,
            tc.tile_pool(name="attnsb", bufs=3) as attnsb_pool,
            tc.tile_pool(name="small", bufs=4) as small_pool,
            tc.tile_pool(name="psA", bufs=2, space="PSUM") as psA,   # stn
            tc.tile_pool(name="psB", bufs=2, space="PSUM") as psB,   # ent
            tc.tile_pool(name="psC", bufs=1, space="PSUM") as psC,   # pt
            tc.tile_pool(name="psD", bufs=2, space="PSUM") as psD,   # u
            tc.tile_pool(name="psE", bufs=1, space="PSUM") as psE,   # batch setup
        ):
            # ---- per-core constants ----
            wt_sb = const_pool.tile([128, 2, Dh], F32)      # W^T tiles: (d%128, d//128, k)
            nc.sync.dma_start(out=wt_sb[:], in_=wt.ap().rearrange("(j p) k -> p j k", p=128))
            b_sb = const_pool.tile([128, 1], F32)
            nc.sync.dma_start(out=b_sb[:], in_=bb.ap())
            id_sb = const_pool.tile([128, 128], F32)
            nc.sync.dma_start(out=id_sb[:], in_=ident.ap())
            id_sb_r = const_pool.tile([128, 128], F32R)
            nc.sync.dma_start(out=id_sb_r[:], in_=ident.ap().bitcast(F32R))
            mneg_sb = const_pool.tile([T, BC], F32)         # bias per (t, b)
            nc.sync.dma_start(out=mneg_sb[:], in_=maskneg.ap().rearrange("b t -> t b"))
            wf_sb = const_pool.tile([128, BC, 2, T], F32)   # (d%128, b, d//128, t)
            nc.sync.dma_start(out=wf_sb[:], in_=wf.ap().rearrange("b (j p) t -> p b j t", p=128))
            reps_sb = const_pool.tile([1, 1], I32)
            nc.sync.dma_start(out=reps_sb[:], in_=reps.ap())
            n_reps = nc.values_load(reps_sb[0:1, 0:1], min_val=1, max_val=1 << 20,
                                    skip_runtime_bounds_check=True)

            import contextlib
            loop_cm = tc.For_i(0, n_reps, 1) if loop else contextlib.nullcontext()
            with loop_cm:
                for b in range(BC):
                    # ---- values = W @ wf[b] + bias ----
                    vals_ps = psE.tile([128, T], F32, tag="bsetup")
                    nc.tensor.matmul(vals_ps[:], wt_sb[:, 0, :], wf_sb[:, b, 0, :],
                                     start=True, stop=False)
                    nc.tensor.matmul(vals_ps[:], wt_sb[:, 1, :], wf_sb[:, b, 1, :],
                                     start=False, stop=True)
                    values_sb = small_pool.tile([128, T], F32, tag="values")
                    nc.scalar.activation(values_sb[:], vals_ps[:],
                                         mybir.ActivationFunctionType.Identity,
                                         bias=b_sb[:])
                    # V^T (fp32r) for the attn matmul
                    vt_ps = psE.tile([T, 128], F32, tag="bsetup")
                    nc.tensor.matmul(vt_ps[:], values_sb[:], id_sb[:],
                                     is_transpose=True, start=True, stop=True)
                    vt_sb = small_pool.tile([T, 128], F32R, tag="vt")
                    nc.scalar.copy(vt_sb[:], vt_ps[:])

                    for ci in range(N // QCHUNK):
                        GB = QCHUNK // NB  # bands per group (4)
                        q_sb = q_pool.tile([128, QCHUNK], F32)
                        nc.sync.dma_start(out=q_sb[:],
                                          in_=img.ap()[b, :, ci * QCHUNK:(ci + 1) * QCHUNK])
                        # accumulation tiles covering the whole 4-band group
                        pnt_sb = pnt_pool.tile([128, GB, 4, T], F32R)
                        attn_sb = attnsb_pool.tile([128, GB, NB], F32)
                        for bi in range(GB):
                            band = ci * GB + bi
                            n0 = band * NB
                            # ---- S^T = values^T(k,t) . q(k,n), fp32 ----
                            stn_ps = psA.tile([T, NB], F32)
                            nc.tensor.matmul(stn_ps[:], values_sb[:],
                                             q_sb[:, bi * NB:(bi + 1) * NB],
                                             start=True, stop=True)
                            # ---- E^T = exp(S^T + maskbias) ----
                            etn_sb = etn_pool.tile([T, NB], F32)
                            nc.scalar.activation(etn_sb[:], stn_ps[:],
                                                 mybir.ActivationFunctionType.Exp,
                                                 bias=mneg_sb[:, b:b + 1])
                            # ---- transpose E^T -> E [128, 4, 32] ----
                            ent_ps = psB.tile([128, 4, T], F32)
                            for j in range(4):
                                nc.tensor.matmul(ent_ps[:, j, :],
                                                 etn_sb[:, j * 128:(j + 1) * 128],
                                                 id_sb[0:T, 0:T],
                                                 is_transpose=True, start=True, stop=True)
                            # ---- row softmax ----
                            sums = small_pool.tile([128, 4, 1], F32, tag="sums")
                            nc.vector.reduce_sum(sums[:], ent_ps[:], axis=mybir.AxisListType.X)
                            r = small_pool.tile([128, 4, 1], F32, tag="recip")
                            nc.vector.reciprocal(r[:], sums[:])
                            nc.vector.tensor_mul(pnt_sb[:, bi, :, :], ent_ps[:],
                                                 r[:].to_broadcast((128, 4, T)))
                            # ---- transpose P -> P^T [32, 512] (fp32r) ----
                            pt_ps = psC.tile([T, NB], F32R)
                            for j in range(4):
                                nc.tensor.matmul(pt_ps[:, j * 128:(j + 1) * 128],
                                                 pnt_sb[:, bi, j, :], id_sb_r[:],
                                                 is_transpose=True, start=True, stop=True)
                            pt_sb = ptsb_pool.tile([T, NB], F32R)
                            nc.scalar.copy(pt_sb[:], pt_ps[:])
                            # ---- U = V^T . P^T (fp32r) -> attn ----
                            u_ps = psD.tile([128, NB], F32)
                            nc.tensor.matmul(u_ps[:], vt_sb[:], pt_sb[:],
                                             start=True, stop=True)
                            nc.vector.tensor_copy(attn_sb[:, bi, :], u_ps[:])
                        # ---- grouped output DMAs (4 bands each) ----
                        g0 = ci * QCHUNK
                        nc.sync.dma_start(
                            out=coeff.ap()[b, g0:g0 + QCHUNK, :].rearrange(
                                "(g j p) t -> p g j t", p=128, j=4),
                            in_=pnt_sb[:].bitcast(F32))
                        nc.sync.dma_start(out=attn.ap()[b, :, g0:g0 + QCHUNK],
                                          in_=attn_sb[:])

    nc.compile()
    return nc


_NC_CACHE = None


def _get_nc():
    global _NC_CACHE
    if _NC_CACHE is None:
        _NC_CACHE = build()
    return _NC_CACHE


def _run(inputs, trace=False, reps=1):
    word_features = np.ascontiguousarray(np.asarray(inputs["word_features"], dtype=np.float32))
    image_features = np.ascontiguousarray(np.asarray(inputs["image_features"], dtype=np.float32))
    words_mask = np.asarray(inputs["words_mask"])
    W = np.asarray(inputs["W"], dtype=np.float32)
    b = np.asarray(inputs["b"], dtype=np.float32)

    img_full = image_features.reshape(B, Dh, N)
    maskneg_full = np.where(words_mask != 0, np.float32(-1e30), np.float32(0.0)).astype(np.float32)
    wt = np.ascontiguousarray(W.T)
    bb = np.ascontiguousarray(b.reshape(Dh, 1))
    ident = np.eye(128, dtype=np.float32)
    reps_arr = np.array([[reps]], dtype=np.int32)

    in_maps = []
    for c in range(NCORES):
        sl = slice(c * BC, (c + 1) * BC)
        in_maps.append({
            "img": np.ascontiguousarray(img_full[sl]),
            "wf": np.ascontiguousarray(word_features[sl]),
            "wt": wt,
            "bb": bb,
            "maskneg": np.ascontiguousarray(maskneg_full[sl]),
            "ident": ident,
            "reps": reps_arr,
        })

    nc = _get_nc()
    res = bass_utils.run_bass_kernel_spmd(nc, in_maps, core_ids=list(range(NCORES)),
                                          trace=trace)
    attn = np.concatenate([r["attn"] for r in res.results], axis=0)
    coeff = np.concatenate([r["coeff"] for r in res.results], axis=0)
    return (attn, coeff), res


def kernel(**inputs):
    (attn, coeff), _ = _run(inputs)
    return attn, coeff


# revision 8
# speedup vs baseline: 26.7125x; 1.1477x over previous
"""Bass/Tile kernel for sparse_attention (nn_Attention_78889959293500) on 8 trn2 cores.

Math (per batch b):
    values = W @ word_features[b] + bias            # [128, 32]
    S[n, t] = sum_k image[b, k, n] * values[k, t]   # [16384, 32]
    S = where(words_mask, -inf, S)
    P = softmax(S, axis=t)                          # [16384, 32]  -> coeff output
    attn[k, n] = sum_t values[k, t] * P[n, t]       # [128, 16384] -> attn output

Sharding: data-parallel over batch B=32 across 8 cores (4 batches each).

Per-core pipeline, processed in groups of 4 bands (4 x 512 = 2048 n):
    S^T 4-stack [128,512] = 4 col-tiled fp32 matmuls (tile_position=(0,32g)):
                            band g's S^T [32,512] lands at partitions 32g..32g+31.
    E4 [128,512]          = one ACT exp over the whole stack (mask folded into the
                            per-partition bias, replicated x4; no max-subtraction:
                            |S|max ~65 < 88).
    E_nt [128,4j,4g,32]   = 4 plain PE transposes of 128-column chunks; each
                            transpose moves 4 bands' worth at once.
    sums/recip/P          = one DVE reduce + reciprocal + mul over the group.
    coeff                 <- grouped DMA of P (f32r bits = valid f32).
    P^T [32,512] per band = 4 plain PE transposes (f32r), ACT-copy to SBUF.
    U [128,512] per band  = single f32r matmul (lhsT=V^T), DVE-copy, grouped DMA.

The whole body sits in a runtime-count For_i ("reps" input, =1 normally) so the
dev harness can time N in-NEFF iterations.
"""

import contextlib

import numpy as np

import concourse.bacc as bacc
import concourse.bass as bass
import concourse.tile as tile
from concourse import mybir
from concourse import bass_utils

B, D, T = 32, 256, 32
Dh, N = 128, 128 * 128
NCORES = 8
BC = B // NCORES          # batches per core
NB = 512                  # n-band size
GB = 4                    # bands per group
NG = N // (NB * GB)       # 8 groups per batch
F32 = mybir.dt.float32
F32R = mybir.dt.float32r
I32 = mybir.dt.int32


def build(loop=True):
    nc = bacc.Bacc("TRN2", target_bir_lowering=False, debug=False, num_devices=NCORES)

    img = nc.dram_tensor("img", (BC, Dh, N), F32, kind="ExternalInput")
    wf = nc.dram_tensor("wf", (BC, D, T), F32, kind="ExternalInput")
    wt = nc.dram_tensor("wt", (D, Dh), F32, kind="ExternalInput")        # W transposed
    bb = nc.dram_tensor("bb", (Dh, 1), F32, kind="ExternalInput")
    maskneg4 = nc.dram_tensor("maskneg4", (BC, 128), F32, kind="ExternalInput")
    ident = nc.dram_tensor("ident", (128, 128), F32, kind="ExternalInput")
    reps = nc.dram_tensor("reps", (1, 1), I32, kind="ExternalInput")

    attn = nc.dram_tensor("attn", (BC, Dh, N), F32, kind="ExternalOutput")
    coeff = nc.dram_tensor("coeff", (BC, N, T), F32, kind="ExternalOutput")

    with tile.TileContext(nc) as tc:
        with (
            tc.tile_pool(name="const", bufs=1) as const_pool,
            tc.tile_pool(name="qin", bufs=3) as q_pool,
            tc.tile_pool(name="e4", bufs=3) as e4_pool,
            tc.tile_pool(name="pnt", bufs=3) as pnt_pool,
            tc.tile_pool(name="ptsb", bufs=3) as ptsb_pool,
            tc.tile_pool(name="attnsb", bufs=2) as attnsb_pool,
            tc.tile_pool(name="small", bufs=4) as small_pool,
            tc.tile_pool(name="psS", bufs=2, space="PSUM") as psS,   # stacked S^T
            tc.tile_pool(name="psE", bufs=2, space="PSUM") as psE,   # stacked E_nt
            tc.tile_pool(name="psP", bufs=1, space="PSUM") as psP,   # flat P^T + setup
            tc.tile_pool(name="psU", bufs=2, space="PSUM") as psU,   # U
        ):
            # ---- per-core constants ----
            wt_sb = const_pool.tile([128, 2, Dh], F32)
            nc.sync.dma_start(out=wt_sb[:], in_=wt.ap().rearrange("(j p) k -> p j k", p=128))
            b_sb = const_pool.tile([128, 1], F32)
            nc.sync.dma_start(out=b_sb[:], in_=bb.ap())
            id_sb = const_pool.tile([128, 128], F32)
            nc.sync.dma_start(out=id_sb[:], in_=ident.ap())
            id_sb_r = const_pool.tile([128, 128], F32R)
            nc.sync.dma_start(out=id_sb_r[:], in_=ident.ap().bitcast(F32R))
            mneg_sb = const_pool.tile([128, BC], F32)   # bias per (32g+t, b)
            nc.sync.dma_start(out=mneg_sb[:], in_=maskneg4.ap().rearrange("b t -> t b"))
            wf_sb = const_pool.tile([128, BC, 2, T], F32)
            nc.sync.dma_start(out=wf_sb[:], in_=wf.ap().rearrange("b (j p) t -> p b j t", p=128))
            reps_sb = const_pool.tile([1, 1], I32)
            nc.sync.dma_start(out=reps_sb[:], in_=reps.ap())
            n_reps = nc.values_load(reps_sb[0:1, 0:1], min_val=1, max_val=1 << 20,
                                    skip_runtime_bounds_check=True)

            loop_cm = tc.For_i(0, n_reps, 1) if loop else contextlib.nullcontext()
            with loop_cm:
                for b in range(BC):
                    # ---- values = W @ wf[b] + bias ----
                    vals_ps = psP.tile([128, T], F32, tag="ptflat")
                    nc.tensor.matmul(vals_ps[:], wt_sb[:, 0, :], wf_sb[:, b, 0, :],
                                     start=True, stop=False)
                    nc.tensor.matmul(vals_ps[:], wt_sb[:, 1, :], wf_sb[:, b, 1, :],
                                     start=False, stop=True)
                    values_sb = small_pool.tile([128, T], F32, tag="values")
                    nc.scalar.activation(values_sb[:], vals_ps[:],
                                         mybir.ActivationFunctionType.Identity,
                                         bias=b_sb[:])
                    vt_ps = psP.tile([T, 128], F32, tag="ptflat")
                    nc.tensor.matmul(vt_ps[:], values_sb[:], id_sb[:],
                                     is_transpose=True, start=True, stop=True)
                    vt_sb = small_pool.tile([T, 128], F32R, tag="vt")
                    nc.scalar.copy(vt_sb[:], vt_ps[:])

                    for ci in range(NG):
                        n_base = ci * NB * GB
                        q_sb = q_pool.tile([128, NB * GB], F32)
                        nc.sync.dma_start(out=q_sb[:],
                                          in_=img.ap()[b, :, n_base:n_base + NB * GB])
                        # ---- col-tiled stacked S^T: band g -> partitions 32g ----
                        s4_ps = psS.tile([128, NB], F32)
                        for g in range(GB):
                            nc.tensor.matmul(s4_ps[32 * g:32 * (g + 1), :],
                                             values_sb[:],
                                             q_sb[:, g * NB:(g + 1) * NB],
                                             start=True, stop=True,
                                             tile_position=(0, 32 * g),
                                             skip_group_check=True)
                        # ---- one exp over the 4-band stack ----
                        e4_sb = e4_pool.tile([128, NB], F32)
                        nc.scalar.activation(e4_sb[:], s4_ps[:],
                                             mybir.ActivationFunctionType.Exp,
                                             bias=mneg_sb[:, b:b + 1])
                        # ---- 4 stacked transposes -> E_nt [128, (g, j, t)] ----
                        ent_ps = psE.tile([128, GB, GB, T], F32)
                        for j in range(GB):
                            nc.tensor.matmul(ent_ps[:, :, j, :],
                                             e4_sb[:, j * 128:(j + 1) * 128],
                                             id_sb[:],
                                             is_transpose=True, start=True, stop=True,
                                             skip_group_check=True)
                        # ---- group softmax ----
                        sums = small_pool.tile([128, GB * GB, 1], F32, tag="sums")
                        nc.vector.reduce_sum(
                            sums[:],
                            ent_ps[:].rearrange("p g j t -> p (g j) t"),
                            axis=mybir.AxisListType.X)
                        r = small_pool.tile([128, GB * GB, 1], F32, tag="recip")
                        nc.vector.reciprocal(r[:], sums[:])
                        pnt_sb = pnt_pool.tile([128, GB, GB, T], F32R)
                        nc.vector.tensor_mul(
                            pnt_sb[:].rearrange("p g j t -> p (g j) t"),
                            ent_ps[:].rearrange("p g j t -> p (g j) t"),
                            r[:].to_broadcast((128, GB * GB, T)))
                        # ---- coeff DMA: n = n_base + 512 g + 128 j + p ----
                        nc.sync.dma_start(
                            out=coeff.ap()[b, n_base:n_base + NB * GB, :].rearrange(
                                "(g j p) t -> p g j t", p=128, j=GB),
                            in_=pnt_sb[:].bitcast(F32))
                        # ---- per band: flat P^T + single U ----
                        attn_sb = attnsb_pool.tile([128, GB, NB], F32)
                        for g in range(GB):
                            pt_ps = psP.tile([T, NB], F32R, tag="ptflat", name=f"pt_{b}_{ci}_{g}")
                            for j in range(GB):
                                nc.tensor.matmul(pt_ps[:, j * 128:(j + 1) * 128],
                                                 pnt_sb[:, g, j, :], id_sb_r[:],
                                                 is_transpose=True, start=True, stop=True,
                                                 skip_group_check=True)
                            pt_sb = ptsb_pool.tile([T, NB], F32R)
                            nc.scalar.copy(pt_sb[:], pt_ps[:])
                            u_ps = psU.tile([128, NB], F32)
                            nc.tensor.matmul(u_ps[:], vt_sb[:], pt_sb[:],
                                             start=True, stop=True)
                            nc.vector.tensor_copy(attn_sb[:, g, :], u_ps[:])
                        nc.sync.dma_start(out=attn.ap()[b, :, n_base:n_base + NB * GB],
                                          in_=attn_sb[:])

    nc.compile()
    return nc


_NC_CACHE = None


def _get_nc():
    global _NC_CACHE
    if _NC_CACHE is None:
        _NC_CACHE = build()
    return _NC_CACHE


def _run(inputs, trace=False, reps=1):
    word_features = np.ascontiguousarray(np.asarray(inputs["word_features"], dtype=np.float32))
    image_features = np.ascontiguousarray(np.asarray(inputs["image_features"], dtype=np.float32))
    words_mask = np.asarray(inputs["words_mask"])
    W = np.asarray(inputs["W"], dtype=np.float32)
    b = np.asarray(inputs["b"], dtype=np.float32)

    img_full = image_features.reshape(B, Dh, N)
    maskneg = np.where(words_mask != 0, np.float32(-1e30), np.float32(0.0)).astype(np.float32)
    maskneg4 = np.tile(maskneg, (1, 4))          # [B, 128]
    wt = np.ascontiguousarray(W.T)
    bb = np.ascontiguousarray(b.reshape(Dh, 1))
    ident = np.eye(128, dtype=np.float32)
    reps_arr = np.array([[reps]], dtype=np.int32)

    in_maps = []
    for c in range(NCORES):
        sl = slice(c * BC, (c + 1) * BC)
        in_maps.append({
            "img": np.ascontiguousarray(img_full[sl]),
            "wf": np.ascontiguousarray(word_features[sl]),
            "wt": wt,
            "bb": bb,
            "maskneg4": np.ascontiguousarray(maskneg4[sl]),
            "ident": ident,
            "reps": reps_arr,
        })

    nc = _get_nc()
    res = bass_utils.run_bass_kernel_spmd(nc, in_maps, core_ids=list(range(NCORES)),
                                          trace=trace)
    attn = np.concatenate([r["attn"] for r in res.results], axis=0)
    coeff = np.concatenate([r["coeff"] for r in res.results], axis=0)
    return (attn, coeff), res


def kernel(**inputs):
    (attn, coeff), _ = _run(inputs)
    return attn, coeff


# revision 9
# speedup vs baseline: 32.2621x; 1.2078x over previous
"""Bass/Tile kernel for sparse_attention (nn_Attention_78889959293500) on 8 trn2 cores.

Math (per batch b):
    values = W @ word_features[b] + bias            # [128, 32]
    S[n, t] = sum_k image[b, k, n] * values[k, t]   # [16384, 32]
    S = where(words_mask, -inf, S)
    P = softmax(S, axis=t)                          # [16384, 32]  -> coeff output
    attn[k, n] = sum_t values[k, t] * P[n, t]       # [128, 16384] -> attn output

Sharding: data-parallel over batch B=32 across 8 cores (4 batches each).

Per-core pipeline, processed in groups of 4 bands (4 x 512 = 2048 n):
    S^T 4-stack [128,512] = 4 col-tiled fp32 matmuls (tile_position=(0,32g)):
                            band g's S^T [32,512] lands at partitions 32g..32g+31.
    E4 [128,512]          = one ACT exp over the whole stack (mask folded into the
                            per-partition bias, replicated x4; no max-subtraction:
                            |S|max ~65 < 88).
    E_nt [128,4j,4g,32]   = 4 plain PE transposes of 128-column chunks; each
                            transpose moves 4 bands' worth at once.
    sums/recip/P          = one DVE reduce + reciprocal + mul over the group.
    coeff                 <- grouped DMA of P (f32r bits = valid f32).
    P^T [32,512] per band = 4 plain PE transposes (f32r), ACT-copy to SBUF.
    U [128,512] per band  = single f32r matmul (lhsT=V^T), DVE-copy, grouped DMA.

The whole body sits in a runtime-count For_i ("reps" input, =1 normally) so the
dev harness can time N in-NEFF iterations.
"""

import contextlib

import numpy as np

import concourse.bacc as bacc
import concourse.bass as bass
import concourse.tile as tile
from concourse import mybir
from concourse import bass_utils

B, D, T = 32, 256, 32
Dh, N = 128, 128 * 128
NCORES = 8
BC = B // NCORES          # batches per core
NB = 512                  # n-band size
GB = 4                    # bands per group
NG = N // (NB * GB)       # 8 groups per batch
F32 = mybir.dt.float32
F32R = mybir.dt.float32r
I32 = mybir.dt.int32


def build(loop=True):
    nc = bacc.Bacc("TRN2", target_bir_lowering=False, debug=False, num_devices=NCORES)

    img = nc.dram_tensor("img", (BC, Dh, N), F32, kind="ExternalInput")
    wf = nc.dram_tensor("wf", (BC, D, T), F32, kind="ExternalInput")
    wt = nc.dram_tensor("wt", (D, Dh), F32, kind="ExternalInput")        # W transposed
    bb = nc.dram_tensor("bb", (Dh, 1), F32, kind="ExternalInput")
    maskneg4 = nc.dram_tensor("maskneg4", (BC, 128), F32, kind="ExternalInput")
    ident = nc.dram_tensor("ident", (128, 128), F32, kind="ExternalInput")
    reps = nc.dram_tensor("reps", (1, 1), I32, kind="ExternalInput")

    attn = nc.dram_tensor("attn", (BC, Dh, N), F32, kind="ExternalOutput")
    coeff = nc.dram_tensor("coeff", (BC, N, T), F32, kind="ExternalOutput")

    with tile.TileContext(nc) as tc:
        with (
            tc.tile_pool(name="const", bufs=1) as const_pool,
            tc.tile_pool(name="qin", bufs=3) as q_pool,
            tc.tile_pool(name="e4", bufs=3) as e4_pool,
            tc.tile_pool(name="pnt", bufs=3) as pnt_pool,
            tc.tile_pool(name="ptsb", bufs=3) as ptsb_pool,
            tc.tile_pool(name="attnsb", bufs=2) as attnsb_pool,
            tc.tile_pool(name="small", bufs=4) as small_pool,
            tc.tile_pool(name="psS", bufs=2, space="PSUM") as psS,   # stacked S^T
            tc.tile_pool(name="psE", bufs=2, space="PSUM") as psE,   # stacked E_nt
            tc.tile_pool(name="psP", bufs=2, space="PSUM") as psP,   # flat P^T + setup
            tc.tile_pool(name="psU", bufs=2, space="PSUM") as psU,   # U
        ):
            # ---- per-core constants ----
            wt_sb = const_pool.tile([128, 2, Dh], F32)
            nc.sync.dma_start(out=wt_sb[:], in_=wt.ap().rearrange("(j p) k -> p j k", p=128))
            b_sb = const_pool.tile([128, 1], F32)
            nc.sync.dma_start(out=b_sb[:], in_=bb.ap())
            id_sb = const_pool.tile([128, 128], F32)
            nc.sync.dma_start(out=id_sb[:], in_=ident.ap())
            id_sb_r = const_pool.tile([128, 128], F32R)
            nc.sync.dma_start(out=id_sb_r[:], in_=ident.ap().bitcast(F32R))
            mneg_sb = const_pool.tile([128, BC], F32)   # bias per (32g+t, b)
            nc.sync.dma_start(out=mneg_sb[:], in_=maskneg4.ap().rearrange("b t -> t b"))
            wf_sb = const_pool.tile([128, BC, 2, T], F32)
            nc.sync.dma_start(out=wf_sb[:], in_=wf.ap().rearrange("b (j p) t -> p b j t", p=128))
            reps_sb = const_pool.tile([1, 1], I32)
            nc.sync.dma_start(out=reps_sb[:], in_=reps.ap())
            n_reps = nc.values_load(reps_sb[0:1, 0:1], min_val=1, max_val=1 << 20,
                                    skip_runtime_bounds_check=True)

            loop_cm = tc.For_i(0, n_reps, 1) if loop else contextlib.nullcontext()
            with loop_cm:
                for b in range(BC):
                    # ---- values = W @ wf[b] + bias ----
                    vals_ps = psP.tile([128, T], F32, tag="ptflat")
                    nc.tensor.matmul(vals_ps[:], wt_sb[:, 0, :], wf_sb[:, b, 0, :],
                                     start=True, stop=False)
                    nc.tensor.matmul(vals_ps[:], wt_sb[:, 1, :], wf_sb[:, b, 1, :],
                                     start=False, stop=True)
                    values_sb = small_pool.tile([128, T], F32, tag="values")
                    nc.scalar.activation(values_sb[:], vals_ps[:],
                                         mybir.ActivationFunctionType.Identity,
                                         bias=b_sb[:])
                    vt_ps = psP.tile([T, 128], F32, tag="ptflat")
                    nc.tensor.matmul(vt_ps[:], values_sb[:], id_sb[:],
                                     is_transpose=True, start=True, stop=True)
                    vt_sb = small_pool.tile([T, 128], F32R, tag="vt")
                    nc.scalar.copy(vt_sb[:], vt_ps[:])

                    for ci in range(NG):
                        n_base = ci * NB * GB
                        q_sb = q_pool.tile([128, NB * GB], F32)
                        nc.sync.dma_start(out=q_sb[:],
                                          in_=img.ap()[b, :, n_base:n_base + NB * GB])
                        # ---- col-tiled stacked S^T: band g -> partitions 32g ----
                        s4_ps = psS.tile([128, NB], F32)
                        for g in range(GB):
                            nc.tensor.matmul(s4_ps[32 * g:32 * (g + 1), :],
                                             values_sb[:],
                                             q_sb[:, g * NB:(g + 1) * NB],
                                             start=True, stop=True,
                                             tile_position=(0, 32 * g),
                                             skip_group_check=True)
                        # ---- one exp over the 4-band stack ----
                        e4_sb = e4_pool.tile([128, NB], F32)
                        nc.scalar.activation(e4_sb[:], s4_ps[:],
                                             mybir.ActivationFunctionType.Exp,
                                             bias=mneg_sb[:, b:b + 1])
                        # ---- 4 stacked transposes -> E_nt [128, (g, j, t)] ----
                        ent_ps = psE.tile([128, GB, GB, T], F32)
                        for j in range(GB):
                            nc.tensor.matmul(ent_ps[:, :, j, :],
                                             e4_sb[:, j * 128:(j + 1) * 128],
                                             id_sb[:],
                                             is_transpose=True, start=True, stop=True,
                                             skip_group_check=True)
                        # ---- group softmax ----
                        sums = small_pool.tile([128, GB * GB, 1], F32, tag="sums")
                        nc.vector.reduce_sum(
                            sums[:],
                            ent_ps[:].rearrange("p g j t -> p (g j) t"),
                            axis=mybir.AxisListType.X)
                        r = small_pool.tile([128, GB * GB, 1], F32, tag="recip")
                        nc.vector.reciprocal(r[:], sums[:])
                        pnt_sb = pnt_pool.tile([128, GB, GB, T], F32R)
                        nc.vector.tensor_mul(
                            pnt_sb[:].rearrange("p g j t -> p (g j) t"),
                            ent_ps[:].rearrange("p g j t -> p (g j) t"),
                            r[:].to_broadcast((128, GB * GB, T)))
                        # ---- coeff DMA: n = n_base + 512 g + 128 j + p ----
                        nc.sync.dma_start(
                            out=coeff.ap()[b, n_base:n_base + NB * GB, :].rearrange(
                                "(g j p) t -> p g j t", p=128, j=GB),
                            in_=pnt_sb[:].bitcast(F32))
                        # ---- per band: flat P^T + single U ----
                        attn_sb = attnsb_pool.tile([128, GB, NB], F32)
                        for g in range(GB):
                            pt_ps = psP.tile([T, NB], F32R, tag="ptflat", name=f"pt_{b}_{ci}_{g}")
                            for j in range(GB):
                                nc.tensor.matmul(pt_ps[:, j * 128:(j + 1) * 128],
                                                 pnt_sb[:, g, j, :], id_sb_r[:],
                                                 is_transpose=True, start=True, stop=True,
                                                 skip_group_check=True)
                            pt_sb = ptsb_pool.tile([T, NB], F32R)
                            nc.scalar.copy(pt_sb[:], pt_ps[:])
                            u_ps = psU.tile([128, NB], F32)
                            nc.tensor.matmul(u_ps[:], vt_sb[:], pt_sb[:],
                                             start=True, stop=True)
                            nc.vector.tensor_copy(attn_sb[:, g, :], u_ps[:])
                        nc.sync.dma_start(out=attn.ap()[b, :, n_base:n_base + NB * GB],
                                          in_=attn_sb[:])

    nc.compile()
    return nc


_NC_CACHE = None


def _get_nc():
    global _NC_CACHE
    if _NC_CACHE is None:
        _NC_CACHE = build()
    return _NC_CACHE


def _run(inputs, trace=False, reps=1):
    word_features = np.ascontiguousarray(np.asarray(inputs["word_features"], dtype=np.float32))
    image_features = np.ascontiguousarray(np.asarray(inputs["image_features"], dtype=np.float32))
    words_mask = np.asarray(inputs["words_mask"])
    W = np.asarray(inputs["W"], dtype=np.float32)
    b = np.asarray(inputs["b"], dtype=np.float32)

    img_full = image_features.reshape(B, Dh, N)
    maskneg = np.where(words_mask != 0, np.float32(-1e30), np.float32(0.0)).astype(np.float32)
    maskneg4 = np.tile(maskneg, (1, 4))          # [B, 128]
    wt = np.ascontiguousarray(W.T)
    bb = np.ascontiguousarray(b.reshape(Dh, 1))
    ident = np.eye(128, dtype=np.float32)
    reps_arr = np.array([[reps]], dtype=np.int32)

    in_maps = []
    for c in range(NCORES):
        sl = slice(c * BC, (c + 1) * BC)
        in_maps.append({
            "img": np.ascontiguousarray(img_full[sl]),
            "wf": np.ascontiguousarray(word_features[sl]),
            "wt": wt,
            "bb": bb,
            "maskneg4": np.ascontiguousarray(maskneg4[sl]),
            "ident": ident,
            "reps": reps_arr,
        })

    nc = _get_nc()
    res = bass_utils.run_bass_kernel_spmd(nc, in_maps, core_ids=list(range(NCORES)),
                                          trace=trace)
    attn = np.concatenate([r["attn"] for r in res.results], axis=0)
    coeff = np.concatenate([r["coeff"] for r in res.results], axis=0)
    return (attn, coeff), res


def kernel(**inputs):
    (attn, coeff), _ = _run(inputs)
    return attn, coeff
